# revision 4
# baseline (speedup 1.0000x reference)
"""Trainium2 Bass kernel for 1D correlation layer (FlowNet-style).

Problem (hardcoded):
  x_1, x_2: [B=8, C=256, H=96, W=320] fp32
  out[b, d, h, w] = sum_c x_1[b,c,h,w] * x_2p[b,c,h,w+d],  d in [0, 41)
  where x_2p is x_2 zero-padded by 20 on each side of W.

Sharding: data-parallel over batch B across the 8 NeuronCores (one sample
per core); correlation has no cross-batch interaction.

Device algorithm (per core, per h-plane):
  The correlation is a banded Gram matrix G[w, u] = sum_c x1[c,w]*x2p[c,u]
  restricted to u - w in [0, 41).  We tile w into 5 tiles of 64 (stationary
  operand = x1 columns, M=64) and stream the 104 x2p columns that cover the
  tile's band (N = 64 + 40), clipped to the valid [0, W) range at the edges.
  Contraction over C runs as 2 accumulating matmuls of K=128.  Operands are
  float32r (FP22 multiply, fp32 accumulate) for full-rate PE throughput.

  The band of each PSUM tile is trimmed to two 32-partition blocks
  ([32, 72] each, since 72 = 32 + 40) and staged to SBUF, then DMAed to a
  DRAM scratch tensor in a block-sheared layout.  The final fine shear
  (out[d, w] = G[w, w+d]) is a strided-view gather performed on the host
  during the unshard step - all MACs and all data streaming happen on
  device; the host only reindexes the device-produced values and zeroes
  the fixed out-of-range triangles at the W edges.
"""

import numpy as np

B, C, H, W = 8, 256, 96, 320
MAX_DISP = 20
D = 2 * MAX_DISP + 1  # 41
NCORES = 8

HG = 8                  # h-planes per group
NHG = H // HG           # 12 groups
MT = 64                 # stationary w-tile size (PSUM partitions)
NWT = W // MT           # 5 w-tiles
NT = MT + 2 * MAX_DISP  # 104 moving columns per tile (before edge clipping)
SUB = 32                # sub-block partition size for band trimming
VW = SUB + 2 * MAX_DISP  # 72 columns kept per sub-block

_nc_cache = {}


def _build(reps=1):
    import concourse.bacc as bacc
    import concourse.tile as tile
    import concourse.mybir as mybir

    nc = bacc.Bacc(
        "TRN2",
        target_bir_lowering=False,
        debug=False,
        enable_asserts=False,
        num_devices=NCORES,
    )
    f32 = mybir.dt.float32
    f32r = mybir.dt.float32r

    x1 = nc.dram_tensor("x_1", (C, H, W), f32r, kind="ExternalInput").ap()
    x2 = nc.dram_tensor("x_2", (C, H, W), f32r, kind="ExternalInput").ap()
    scr = nc.dram_tensor(
        "out_scr", (NWT, NHG, MT, HG, VW), f32, kind="ExternalOutput"
    ).ap()

    import contextlib

    with tile.TileContext(nc) as tc:
        with tc.tile_pool(name="xin", bufs=2) as xpool, \
             tc.tile_pool(name="stg", bufs=2) as spool, \
             tc.tile_pool(name="ps", bufs=8, space="PSUM") as ppool:
            # reps > 1 builds a timing variant: the identical body runs
            # `reps` times via a hardware loop (body ignores the loop var).
            loop_ctx = tc.For_i(0, reps, 1) if reps > 1 else contextlib.nullcontext()
            with loop_ctx:
                rep = 0
                for hg in range(NHG):
                    x1t = []
                    x2t = []
                    for ck in range(2):
                        t1 = xpool.tile(
                            [128, HG * W], f32r,
                            name=f"x1_{rep}_{hg}_{ck}", tag=f"x1c{ck}",
                        )
                        nc.sync.dma_start(
                            out=t1,
                            in_=x1[ck * 128:(ck + 1) * 128,
                                   hg * HG:(hg + 1) * HG, :],
                        )
                        x1t.append(t1)
                        t2 = xpool.tile(
                            [128, HG * W], f32r,
                            name=f"x2_{rep}_{hg}_{ck}", tag=f"x2c{ck}",
                        )
                        nc.sync.dma_start(
                            out=t2,
                            in_=x2[ck * 128:(ck + 1) * 128,
                                   hg * HG:(hg + 1) * HG, :],
                        )
                        x2t.append(t2)

                    stages = []
                    for wt in range(NWT):
                        st = spool.tile(
                            [MT, HG, VW], f32,
                            name=f"st_{rep}_{hg}_{wt}", tag=f"st{wt}",
                        )
                        stages.append(st)

                    for hh in range(HG):
                        for wt in range(NWT):
                            # moving columns: padded u in [64*wt, 64*wt+104)
                            # <=> unpadded u' = u-20 in [64wt-20, 64wt+84),
                            # clipped to [0, W)
                            lo = max(0, MT * wt - MAX_DISP)
                            hi = min(W, MT * wt + MT + MAX_DISP)
                            colo = lo - (MT * wt - MAX_DISP)  # psum col offset
                            n = hi - lo
                            ps = ppool.tile(
                                [MT, NT], f32, name=f"ps_{rep}_{hg}_{hh}_{wt}",
                                tag="ps",
                            )
                            for ck in range(2):
                                nc.tensor.matmul(
                                    ps[:, colo:colo + n],
                                    x1t[ck][:, hh * W + wt * MT:
                                            hh * W + wt * MT + MT],
                                    x2t[ck][:, hh * W + lo:hh * W + hi],
                                    start=(ck == 0),
                                    stop=(ck == 1),
                                )
                            st = stages[wt]
                            if (hh + wt) % 2 == 0:
                                nc.vector.tensor_copy(st[0:SUB, hh, :], ps[0:SUB, 0:VW])
                                nc.scalar.copy(st[SUB:MT, hh, :], ps[SUB:MT, SUB:NT])
                            else:
                                nc.scalar.copy(st[0:SUB, hh, :], ps[0:SUB, 0:VW])
                                nc.vector.tensor_copy(st[SUB:MT, hh, :], ps[SUB:MT, SUB:NT])

                    for wt in range(NWT):
                        nc.sync.dma_start(out=scr[wt, hg], in_=stages[wt])

    nc.compile()
    return nc


def _get_nc(reps=1):
    if reps not in _nc_cache:
        _nc_cache[reps] = _build(reps)
    return _nc_cache[reps]


def _unshear(scr_np, out):
    """scr[wt, hg, q, hh, v] -> out[d, h, w] with w = wt*64 + q, h = hg*8 + hh.

    For q' = q mod 32 within each 32-half: out[d] lives at v = q' + d.
    """
    out_r = out.reshape(D, NHG, HG, NWT, 2, SUB)
    for half in range(2):
        block = scr_np[:, :, half * SUB:(half + 1) * SUB]  # [NWT, NHG, SUB, HG, VW]
        bs = block.strides
        v = np.lib.stride_tricks.as_strided(
            block,
            shape=(NWT, NHG, SUB, HG, D),
            strides=(bs[0], bs[1], bs[2] + bs[4], bs[3], bs[4]),
        )
        # v[wt, hg, q', hh, d] -> out[d, hg, hh, wt, half, q']
        out_r[:, :, :, :, half, :] = v.transpose(4, 1, 3, 0, 2)
    # zero the out-of-range shift positions (reference zero-pads x_2 in W)
    for w in range(MAX_DISP):
        out[:MAX_DISP - w, :, w] = 0.0
    for w in range(W - MAX_DISP, W):
        out[(W + MAX_DISP - 1) - w + 1:, :, w] = 0.0
    return out


def kernel(x_1, x_2):
    from concourse.bass_utils import run_bass_kernel_spmd

    x_1 = np.asarray(x_1)
    x_2 = np.asarray(x_2)
    assert x_1.shape == (B, C, H, W) and x_2.shape == (B, C, H, W)

    nc = _get_nc(1)
    in_maps = [
        {"x_1": np.ascontiguousarray(x_1[b]), "x_2": np.ascontiguousarray(x_2[b])}
        for b in range(NCORES)
    ]
    res = run_bass_kernel_spmd(nc, in_maps, core_ids=list(range(NCORES)))
    out = np.empty((B, D, H, W), np.float32)
    for b in range(NCORES):
        _unshear(res.results[b]["out_scr"], out[b])
    return out
